# revision 1
# baseline (speedup 1.0000x reference)
"""Trainium2 Bass kernel for GQA attention with RoPE and block-diagonal
(document) causal masking, sharded over 8 NeuronCores by KV head group.

Per core c (of 8): Q heads 4c..4c+3, KV head c, both batches.
All matmuls in float32r (tf32-grade operand rounding, full PE rate at
moving-dim >= 256, fp32 PSUM accumulation).

v2 layout highlights:
  - x^T half-sequence (8 MB) resident in SBUF; projections run as 6
    sequential 16-matmul accumulation groups per half on a 3-bank PSUM
    rotation, with RoPE (DVE) trailing each group.
  - attention processes heads in PAIRS: one [128,512] scores matmul per
    key tile (multi-dim free AP over the two heads' Q), one exp, one
    [65,512] PV accumulation with an appended ones column for the
    softmax denominators; masking is a 0/1 multiply on partial tiles.
  - o_proj writes through a deep sbuf ring.
Host sums the 8 per-core partials.
"""
import sys
sys.path.insert(0, "/opt/trn_rl_repo")
import numpy as np

B, S, DIM = 2, 2048, 2048
NH, NKV, HD = 32, 8, 64
HPC = NH // 8           # 4 q-heads per core
MLOC = HPC * HD         # 256 local q dims
TQ = 256                # attention query chunk
NCORES = 8
NKC = DIM // 128        # 16 contraction chunks
NTC = S // 128          # 16 token chunks of 128
SCALE = 1.0 / 8.0

_nc_cache = {}


def _schedule(doc_ids):
    """Per batch: for each query chunk, the key-tile band and mask info."""
    doc = np.asarray(doc_ids)
    sched = []
    masks = []
    for b in range(B):
        d = doc[b]
        change = np.empty(S, dtype=np.int64)
        change[0] = 0
        idx = np.arange(1, S)
        change[1:] = np.where(d[1:] != d[:-1], idx, 0)
        start_idx = np.maximum.accumulate(change)
        per_qc = []
        for qc in range(S // TQ):
            q0 = qc * TQ
            t0 = int(start_idx[q0]) // 128
            t1 = (q0 + TQ) // 128
            row = []
            for kt in range(t0, t1):
                k0 = kt * 128
                full = (k0 + 127 <= q0 and d[k0] == d[k0 + 127] == d[q0] == d[q0 + TQ - 1])
                if full:
                    row.append((kt, None))
                else:
                    ks = np.arange(k0, k0 + 128)
                    qs = np.arange(q0, q0 + TQ)
                    m = (d[ks][:, None] == d[qs][None, :]) & (ks[:, None] <= qs[None, :])
                    masks.append(m.astype(np.float32))  # stacked later; cast in _prep
                    row.append((kt, len(masks) - 1))
            per_qc.append(row)
        sched.append(per_qc)
    if not masks:
        masks.append(np.zeros((128, TQ), np.float32))
    import ml_dtypes
    return sched, np.stack(masks).astype(ml_dtypes.bfloat16)


def _build_nc(sched, nmask):
    import concourse.bacc as bacc
    import concourse.mybir as mybir
    import concourse.tile as tile
    from concourse.masks import make_identity

    F32, F32R = mybir.dt.float32, mybir.dt.float32r
    Exp = mybir.ActivationFunctionType.Exp

    nc = bacc.Bacc()
    xT = nc.dram_tensor("xT", (B, DIM, S), F32, kind="ExternalInput")
    wq = nc.dram_tensor("wq", (DIM, MLOC), F32, kind="ExternalInput")
    wkv = nc.dram_tensor("wkv", (DIM, 128), F32, kind="ExternalInput")
    wo = nc.dram_tensor("wo", (MLOC, DIM), F32, kind="ExternalInput")
    cos128 = nc.dram_tensor("cos128", (128, S), F32, kind="ExternalInput")
    sin128 = nc.dram_tensor("sin128", (128, S), F32, kind="ExternalInput")
    masks = nc.dram_tensor("masks", (nmask, 128, TQ), mybir.dt.bfloat16, kind="ExternalInput")
    y = nc.dram_tensor("y", (B, S, DIM), F32, kind="ExternalOutput")

    with tile.TileContext(nc) as tc:
        with (
            tc.tile_pool(name="const", bufs=1) as cst,
            tc.tile_pool(name="xt", bufs=1) as xtp,
            tc.tile_pool(name="big", bufs=1) as big,
            tc.tile_pool(name="rope", bufs=3) as rp,
            tc.tile_pool(name="pt", bufs=4) as ptp,
            tc.tile_pool(name="mask", bufs=2) as mp,
            tc.tile_pool(name="small", bufs=3) as sp,
            tc.tile_pool(name="ysb", bufs=4) as yp,
            tc.tile_pool(name="pp", bufs=3, space="PSUM") as pp,
            tc.tile_pool(name="pa", bufs=5, space="PSUM") as pa,
        ):
            # ---- constants ----
            wq_sb = cst.tile([128, NKC, MLOC], F32R)
            for kc in range(NKC):
                nc.sync.dma_start(
                    wq_sb[:, kc, :], wq[kc * 128:(kc + 1) * 128, :].bitcast(F32R))
            wkv_sb = cst.tile([128, NKC, 128], F32R)
            for kc in range(NKC):
                nc.sync.dma_start(
                    wkv_sb[:, kc, :], wkv[kc * 128:(kc + 1) * 128, :].bitcast(F32R))
            wo_sb = cst.tile([128, 2, DIM], F32R)
            nc.sync.dma_start(wo_sb[:], wo[:].rearrange("(c p) m -> p c m", p=128).bitcast(F32R))
            cos_sb = cst.tile([128, S], F32)
            nc.sync.dma_start(cos_sb[:], cos128[:])
            sin_sb = cst.tile([128, S], F32)
            nc.sync.dma_start(sin_sb[:], sin128[:])
            ident = cst.tile([64, 64], F32)
            make_identity(nc, ident[:])
            scratch1 = cst.tile([128, 1], F32)
            nc.gpsimd.memset(scratch1[:], 1.0)
            ones64 = cst.tile([128, 64], F32R)
            nc.vector.tensor_copy(ones64[:], scratch1[:].broadcast_to([128, 64]))

            for b in range(B):
                # per-head-pair Q^T: [64, 2*S], head 2p+j at cols j*S..
                qrt = [big.tile([64, 2 * S], F32R, tag=f"qrt{m}", name=f"qrt{m}")
                       for m in range(2)]
                krt = big.tile([64, S], F32R, tag="krt")
                vaug = big.tile([128, NTC, 128], F32R, tag="vaug")
                or2t = [big.tile([128, S], F32R, tag=f"or2t{m}", name=f"or2t{m}")
                        for m in range(2)]

                # ================= projections + rope =================
                for tqi in range(4):
                    t0 = tqi * 512
                    tsl = slice(t0, t0 + 512)
                    xt_lo = xtp.tile([128, NKC // 2, 512], F32R, tag="xtlo", name=f"xtlo{b}{tqi}")
                    xt_hi = xtp.tile([128, NKC // 2, 512], F32R, tag="xthi", name=f"xthi{b}{tqi}")
                    for kc in range(NKC):
                        dst = xt_lo if kc < NKC // 2 else xt_hi
                        nc.sync.dma_start(
                            dst[:, kc % (NKC // 2), :],
                            xT[b, kc * 128:(kc + 1) * 128, t0:t0 + 512].bitcast(F32R))
                    for what in ("q0", "q1", "kv"):
                        gps = pp.tile([128, 512], F32, tag="pp", name=f"gps{b}{tqi}{what}")
                        for kc in range(NKC):
                            if what == "q0":
                                lhs = wq_sb[:, kc, 0:128]
                            elif what == "q1":
                                lhs = wq_sb[:, kc, 128:256]
                            else:
                                lhs = wkv_sb[:, kc, :]
                            xsrc = xt_lo if kc < NKC // 2 else xt_hi
                            nc.tensor.matmul(
                                gps[:], lhs, xsrc[:, kc % (NKC // 2), :],
                                start=(kc == 0), stop=(kc == NKC - 1))
                        if what in ("q0", "q1"):
                            m = 0 if what == "q0" else 1
                            gsb = rp.tile([128, 512], F32, tag="gsb")
                            nc.scalar.copy(gsb[:], gps[:])
                            tmp = rp.tile([128, 512], F32, tag="ra")
                            for blk in (0, 64):
                                nc.vector.tensor_mul(
                                    tmp[blk:blk + 32], gsb[blk + 32:blk + 64],
                                    sin_sb[blk + 32:blk + 64, tsl])
                                nc.vector.tensor_mul(
                                    tmp[blk + 32:blk + 64], gsb[blk:blk + 32],
                                    sin_sb[blk:blk + 32, tsl])
                            tmp2 = rp.tile([128, 512], F32, tag="rb")
                            nc.vector.tensor_mul(tmp2[:], gsb[:], cos_sb[:, tsl])
                            nc.vector.tensor_add(
                                qrt[m][:, t0:t0 + 512], tmp[0:64], tmp2[0:64])
                            nc.vector.tensor_add(
                                qrt[m][:, S + t0:S + t0 + 512], tmp[64:128], tmp2[64:128])
                        else:
                            gsb = rp.tile([128, 512], F32, tag="gsb")
                            nc.scalar.copy(gsb[:], gps[:])
                            tmp = rp.tile([64, 512], F32, tag="ra")
                            nc.vector.tensor_mul(tmp[0:32], gsb[32:64], sin_sb[32:64, tsl])
                            nc.vector.tensor_mul(tmp[32:64], gsb[0:32], sin_sb[0:32, tsl])
                            tmp2 = rp.tile([64, 512], F32, tag="rb")
                            nc.vector.tensor_mul(tmp2[:], gsb[0:64], cos_sb[0:64, tsl])
                            nc.vector.tensor_add(krt[:, tsl], tmp[:], tmp2[:])
                            vt = sp.tile([64, 512], F32, tag="vt")
                            nc.scalar.copy(vt[:], gsb[64:128])
                            for tc4 in range(4):
                                kt = (t0 // 128) + tc4
                                ptr = pa.tile([128, 64], F32, tag="pa", name=f"ptr{b}{tqi}{tc4}")
                                nc.tensor.transpose(
                                    ptr[:], vt[:, tc4 * 128:(tc4 + 1) * 128], ident[:])
                                nc.scalar.copy(vaug[:, kt, 0:64], ptr[:])
                                nc.scalar.copy(vaug[:, kt, 64:128], ones64[:])

                # ================= attention (head pairs) =================
                for qc in range(S // TQ):
                    q0 = qc * TQ
                    row = sched[b][qc]
                    mtiles = {}
                    for i, (kt, mi) in enumerate(row):
                        if mi is not None:
                            mt = mp.tile([128, TQ], mybir.dt.bfloat16, tag=f"m{i % 6}", name=f"mt{i}")
                            nc.sync.dma_start(mt[:], masks[mi])
                            mtiles[kt] = mt
                    for pr in range(2):
                        # both heads' Q columns: [64, 2, TQ] multi-dim free AP
                        qpair = qrt[pr][:].rearrange("p (j s) -> p j s", j=2)[:, :, q0:q0 + TQ]
                        o_ps = pa.tile([128, 2 * TQ], F32, tag="pa", name=f"ops{b}{qc}{pr}")
                        for i, (kt, mi) in enumerate(row):
                            s_ps = pa.tile([128, 2 * TQ], F32, tag="pa", name=f"sps{b}{qc}{pr}{i}")
                            nc.tensor.matmul(
                                s_ps[:], krt[:, kt * 128:(kt + 1) * 128], qpair,
                                start=True, stop=True)
                            pt = ptp.tile([128, 2 * TQ], F32R, tag="pt")
                            nc.scalar.activation(pt[:], s_ps[:], Exp, scale=SCALE)
                            if mi is not None:
                                nc.vector.tensor_mul(pt[:, 0:TQ], pt[:, 0:TQ], mtiles[kt][:])
                                nc.vector.tensor_mul(pt[:, TQ:2 * TQ], pt[:, TQ:2 * TQ], mtiles[kt][:])
                            nc.tensor.matmul(
                                o_ps[:], vaug[:, kt, :], pt[:],
                                start=(i == 0), stop=(i == len(row) - 1))
                        zb = sp.tile([64, 2 * TQ], F32, tag="zb")
                        with nc.allow_low_precision(reason="normalization reciprocal"):
                            nc.vector.reciprocal(zb[:], o_ps[64:128, :])
                        qsl = slice(q0, q0 + TQ)
                        nc.vector.tensor_mul(
                            or2t[pr][0:64, qsl], o_ps[0:64, 0:TQ], zb[:, 0:TQ])
                        nc.vector.tensor_mul(
                            or2t[pr][64:128, qsl], o_ps[0:64, TQ:2 * TQ], zb[:, TQ:2 * TQ])
                    # o_proj for this query chunk's token tiles
                    for tc_ in range(qc * (TQ // 128), (qc + 1) * (TQ // 128)):
                        for mc in range(4):
                            y_ps = pa.tile([128, 512], F32, tag="pa", name=f"yps{b}{tc_}{mc}")
                            for hp in range(2):
                                nc.tensor.matmul(
                                    y_ps[:], or2t[hp][:, tc_ * 128:(tc_ + 1) * 128],
                                    wo_sb[:, hp, mc * 512:(mc + 1) * 512],
                                    start=(hp == 0), stop=(hp == 1))
                            y_sb = yp.tile([128, 512], F32, tag="ysb")
                            nc.scalar.copy(y_sb[:], y_ps[:])
                            nc.sync.dma_start(
                                y[b, tc_ * 128:(tc_ + 1) * 128, mc * 512:(mc + 1) * 512],
                                y_sb[:])


    nc.finalize()
    return nc


def _prep_inputs(x, rope_cos, rope_sin, doc_ids, Wq, Wk, Wv, Wo):
    x = np.asarray(x, np.float32)
    xT = np.ascontiguousarray(x.transpose(0, 2, 1))
    cosT = np.asarray(rope_cos, np.float32).T          # (32, S)
    sinT = np.asarray(rope_sin, np.float32).T
    cos128 = np.tile(np.concatenate([cosT, cosT], 0), (2, 1))      # (128, S)
    sin128 = np.tile(np.concatenate([sinT, -sinT], 0), (2, 1))
    sched, masks = _schedule(doc_ids)
    Wq = np.asarray(Wq, np.float32)
    Wk = np.asarray(Wk, np.float32)
    Wv = np.asarray(Wv, np.float32)
    Wo = np.asarray(Wo, np.float32)
    in_maps = []
    for c in range(NCORES):
        wq_c = np.ascontiguousarray(Wq[c * MLOC:(c + 1) * MLOC].T)      # (DIM, 256)
        wk_c = Wk[c * HD:(c + 1) * HD].T                                # (DIM, 64)
        wv_c = Wv[c * HD:(c + 1) * HD].T
        wkv_c = np.ascontiguousarray(np.concatenate([wk_c, wv_c], 1))   # (DIM, 128)
        wo_c = np.ascontiguousarray(Wo[:, c * MLOC:(c + 1) * MLOC].T)   # (256, DIM)
        in_maps.append({
            "xT": xT, "wq": wq_c, "wkv": wkv_c, "wo": wo_c,
            "cos128": cos128, "sin128": sin128, "masks": masks,
        })
    return sched, masks, in_maps


def kernel(x, rope_cos, rope_sin, doc_ids, Wq, Wk, Wv, Wo):
    from concourse.bass_utils import run_bass_kernel_spmd
    sched, masks, in_maps = _prep_inputs(
        x, rope_cos, rope_sin, doc_ids, Wq, Wk, Wv, Wo)
    key = (tuple(tuple(tuple((kt, mi is not None) for kt, mi in row) for row in sb)
                 for sb in sched), masks.shape[0])
    nc = _nc_cache.get(key)
    if nc is None:
        nc = _build_nc(sched, masks.shape[0])
        _nc_cache[key] = nc
    res = run_bass_kernel_spmd(nc, in_maps, core_ids=list(range(NCORES)))
    y = np.zeros((B, S, DIM), np.float32)
    for c in range(NCORES):
        y += res.results[c]["y"]
    return y



# revision 27
# speedup vs baseline: 29.4829x; 29.4829x over previous
"""Trainium2 Bass kernel for GQA attention with RoPE and block-diagonal
(document) causal masking, sharded over 8 NeuronCores by batch x head
quarter: core c handles batch c//4 and q heads 8*(c%4)..8*(c%4)+7 with
kv heads 2*(c%4), 2*(c%4)+1 (whole GQA groups stay local).

v3 layout highlights vs v2:
  - bf16 datapath end to end (x, weights, Q/K/V, probabilities, y
    partials) with f32 PSUM accumulation: halves HBM traffic and
    doubles DVE throughput; matmuls run at 1 cycle/row either way.
  - per-core work is one batch only, so x load and y partial write are
    8.4 MB each instead of 33.5 MB; host sums 4 partials per batch.
  - attention in 128-query chunks with all 4 heads of a GQA group in a
    single matmul pair ([128,512] scores, [128,512] PV with an
    appended ones block for denominators): 35% fewer PE cycles than
    the 256-query head-pair schedule.
  - one union (over both batches) block schedule so the SPMD program
    is identical on all cores; per-batch masks zero out tiles that a
    particular batch does not need.
  - PSUM->SBUF copies balanced across Act and Pool engines.
Host sums the 4 per-core partials of each batch.
"""
import sys
sys.path.insert(0, "/opt/trn_rl_repo")
import numpy as np

B, S, DIM = 2, 2048, 2048
NH, NKV, HD = 32, 8, 64
NCORES = 8
CPB = 4                  # cores per batch
HPC = NH // CPB          # 8 q heads per core
NKVC = NKV // CPB        # 2 kv heads per core
MLOC = HPC * HD          # 512 local q dims
TQ = 128                 # attention query chunk
NKC = DIM // 128         # 16 contraction chunks
NTC = S // 128           # 16 token tiles of 128
SCALE = 1.0 / 8.0

_nc_cache = {}


def _doc_starts(d):
    change = np.zeros(S, dtype=np.int64)
    idx = np.arange(1, S)
    change[1:] = np.where(d[1:] != d[:-1], idx, 0)
    return np.maximum.accumulate(change)


def _schedule(doc_ids):
    """Union (over batches) band schedule for 128-query chunks, plus
    per-batch masks (pre-tiled x4 along the free dim for the 4 heads)."""
    import ml_dtypes
    doc = np.asarray(doc_ids)
    starts = [_doc_starts(doc[b]) for b in range(B)]
    sched = []          # per qc: list of (kt, mask_index or None)
    masks = [[] for _ in range(B)]
    for qc in range(S // TQ):
        q0 = qc * TQ
        t0 = min(int(starts[b][q0]) // 128 for b in range(B))
        row = []
        for kt in range(t0, qc + 1):
            k0 = kt * 128
            full_both = True
            for b in range(B):
                d = doc[b]
                if not (k0 + 127 <= q0 and
                        d[k0] == d[k0 + 127] == d[q0] == d[q0 + TQ - 1]):
                    full_both = False
            if full_both:
                row.append((kt, None))
            else:
                ks = np.arange(k0, k0 + 128)
                qs = np.arange(q0, q0 + TQ)
                for b in range(B):
                    d = doc[b]
                    m = (d[ks][:, None] == d[qs][None, :]) & \
                        (ks[:, None] <= qs[None, :])
                    masks[b].append(np.tile(m.astype(np.float32), (1, 4)))
                row.append((kt, len(masks[0]) - 1))
        sched.append(row)
    if not masks[0]:
        for b in range(B):
            masks[b].append(np.zeros((128, 4 * TQ), np.float32))
    mk = [np.stack(masks[b]).astype(ml_dtypes.bfloat16) for b in range(B)]
    return sched, mk


def _build_nc(sched, nmask):
    import concourse.bacc as bacc
    import concourse.mybir as mybir
    import concourse.tile as tile
    from concourse.masks import make_identity

    F32, BF16 = mybir.dt.float32, mybir.dt.bfloat16
    Exp = mybir.ActivationFunctionType.Exp

    nc = bacc.Bacc()
    xT = nc.dram_tensor("xT", (DIM, S), BF16, kind="ExternalInput")
    wq = nc.dram_tensor("wq", (DIM, MLOC), BF16, kind="ExternalInput")
    wkv = nc.dram_tensor("wkv", (DIM, NKVC * 128), BF16, kind="ExternalInput")
    wo = nc.dram_tensor("wo", (MLOC, DIM), BF16, kind="ExternalInput")
    cos128 = nc.dram_tensor("cos128", (128, S), BF16, kind="ExternalInput")
    sin128 = nc.dram_tensor("sin128", (128, S), BF16, kind="ExternalInput")
    masks = nc.dram_tensor("masks", (nmask, 128, 4 * TQ), BF16,
                           kind="ExternalInput")
    y = nc.dram_tensor("y", (S, DIM), BF16, kind="ExternalOutput")

    with tile.TileContext(nc) as tc:
        with (
            tc.tile_pool(name="const", bufs=1) as cst,
            tc.tile_pool(name="big", bufs=1) as big,
            tc.tile_pool(name="xt", bufs=2) as xtp,
            tc.tile_pool(name="rope", bufs=4) as rp,
            tc.tile_pool(name="vt", bufs=2) as vtp,
            tc.tile_pool(name="mask", bufs=2) as mp,
            tc.tile_pool(name="pt", bufs=4) as ptp,
            tc.tile_pool(name="zb", bufs=2) as zbp,
            tc.tile_pool(name="ysb", bufs=2) as yp,
            tc.tile_pool(name="pp", bufs=2, space="PSUM") as pp,
            tc.tile_pool(name="ps", bufs=2, space="PSUM") as psp,
            tc.tile_pool(name="po", bufs=2, space="PSUM") as pop,
            tc.tile_pool(name="py", bufs=2, space="PSUM") as pyp,
        ):
            # ---- constants; wq/x first pieces lead so PE starts early ----
            wq_sb = cst.tile([128, NKC, MLOC], BF16)
            wkv_sb = cst.tile([128, NKC, NKVC * 128], BF16)
            wo_sb = cst.tile([128, 4, DIM], BF16)
            cos_sb = cst.tile([128, S], BF16)
            sin_sb = cst.tile([128, S], BF16)
            xts = []
            for t in range(4):
                xts.append(xtp.tile([128, NKC, 512], BF16, tag="xt",
                                    name=f"xt{t}"))
            wq_r = wq[:].rearrange("(c p) m -> p c m", p=128)
            x0_r = xT[:, 0:512].rearrange("(c p) t -> p c t", p=128)
            for piece in range(4):
                psl = slice(piece * 4, (piece + 1) * 4)
                nc.sync.dma_start(wq_sb[:, psl, :], wq_r[:, psl, :])
                nc.sync.dma_start(xts[0][:, psl, :], x0_r[:, psl, :])
            nc.sync.dma_start(
                wkv_sb[:], wkv[:].rearrange("(c p) m -> p c m", p=128))
            nc.sync.dma_start(cos_sb[:], cos128[:])
            nc.sync.dma_start(sin_sb[:], sin128[:])
            ident = cst.tile([64, 64], F32)
            make_identity(nc, ident[:])

            # persistent per-core tensors (one batch)
            qrt = big.tile([64, NKVC, 4, S], BF16, tag="qrt")
            krt = big.tile([64, NKVC, S], BF16, tag="krt")
            vaug = big.tile([128, NKVC, NTC, 128], BF16, tag="vaug")
            or2t = big.tile([128, 4, S], BF16, tag="or2t")
            nc.gpsimd.memset(vaug[:, :, :, 64:128], 1.0)

            def o_proj(qc, last=False):
                ysb = yp.tile([128, DIM], BF16, tag="ysb")
                for mc in range(4):
                    y_ps = pyp.tile([128, 512], F32, tag="py",
                                    name=f"yps{qc}{mc}")
                    for hp in range(4):
                        nc.tensor.matmul(
                            y_ps[:],
                            or2t[:, hp, qc * 128:(qc + 1) * 128],
                            wo_sb[:, hp, mc * 512:(mc + 1) * 512],
                            start=(hp == 0), stop=(hp == 3))
                    msl = slice(mc * 512, (mc + 1) * 512)
                    # Pool cannot read PSUM; split PSUM->SBUF casts Act/DVE
                    if mc % 2 == 0:
                        nc.scalar.copy(ysb[:, msl], y_ps[:])
                    else:
                        nc.vector.tensor_copy(ysb[:, msl], y_ps[:])
                    if last:
                        nc.sync.dma_start(
                            y[qc * 128:(qc + 1) * 128, msl], ysb[:, msl])
                    elif mc % 2 == 1:
                        nc.sync.dma_start(
                            y[qc * 128:(qc + 1) * 128,
                              (mc - 1) * 512:(mc + 1) * 512],
                            ysb[:, (mc - 1) * 512:(mc + 1) * 512])

            def load_masks(qc, mtiles_all):
                row = sched[qc]
                mtiles = {}
                for i, (kt, mi) in enumerate(row):
                    if mi is not None:
                        mt = mp.tile([128, 4 * TQ], BF16,
                                     tag=f"m{qc % 2}_{i}", name=f"mt{qc}{i}")
                        nc.sync.dma_start(mt[:], masks[mi])
                        mtiles[kt] = mt
                mtiles_all[qc] = mtiles

            mtiles_all = {}
            for t in range(4):
                ts0 = t * 512
                tsl = slice(ts0, ts0 + 512)
                xt = xts[t]

                # ---------------- projections + rope ----------------
                # w pairs with kc interleaved across the two open PSUM
                # groups: matmuls start as soon as the first x/wq DMA
                # pieces land instead of waiting for a full 16-kc set
                def lhs_of(w, kc):
                    if w < 4:
                        return wq_sb[:, kc, w * 128:(w + 1) * 128]
                    return wkv_sb[:, kc, (w - 4) * 128:(w - 3) * 128]

                def proj_tail(w, gps):
                    gsb = rp.tile([128, 512], BF16, tag="gsb")
                    nc.scalar.copy(gsb[:], gps[:])
                    if w < 4:
                        g, j = w // 2, w % 2
                        tmp = rp.tile([128, 512], BF16, tag="ra")
                        for blk in (0, 64):
                            nc.vector.tensor_mul(
                                tmp[blk:blk + 32], gsb[blk + 32:blk + 64],
                                sin_sb[blk + 32:blk + 64, tsl])
                            nc.vector.tensor_mul(
                                tmp[blk + 32:blk + 64], gsb[blk:blk + 32],
                                sin_sb[blk:blk + 32, tsl])
                        tmp2 = rp.tile([128, 512], BF16, tag="rb")
                        nc.vector.tensor_mul(tmp2[:], gsb[:], cos_sb[:, tsl])
                        nc.vector.tensor_add(
                            qrt[:, g, 2 * j, tsl], tmp[0:64], tmp2[0:64])
                        nc.vector.tensor_add(
                            qrt[:, g, 2 * j + 1, tsl], tmp[64:128],
                            tmp2[64:128])
                    else:
                        # K rope on the (otherwise idle) Pool engine; all
                        # operands are SBUF so gpsimd may touch them
                        g = w - 4
                        tmp = rp.tile([64, 512], BF16, tag="ra")
                        nc.gpsimd.tensor_mul(
                            tmp[0:32], gsb[32:64], sin_sb[32:64, tsl])
                        nc.gpsimd.tensor_mul(
                            tmp[32:64], gsb[0:32], sin_sb[0:32, tsl])
                        tmp2 = rp.tile([64, 512], BF16, tag="rb")
                        nc.gpsimd.tensor_mul(
                            tmp2[:], gsb[0:64], cos_sb[0:64, tsl])
                        nc.gpsimd.tensor_add(krt[:, g, tsl], tmp[:], tmp2[:])
                        vt = vtp.tile([64, 512], F32, tag="vt")
                        nc.scalar.copy(vt[:], gps[64:128])
                        for tc4 in range(4):
                            kt = t * 4 + tc4
                            ptr = pyp.tile([128, 64], F32, tag="py",
                                           name=f"ptr{t}{g}{tc4}")
                            nc.tensor.transpose(
                                ptr[:], vt[:, tc4 * 128:(tc4 + 1) * 128],
                                ident[:])
                            nc.scalar.copy(vaug[:, g, kt, 0:64], ptr[:])

                for w in range(6):
                    gps = pp.tile([128, 512], F32, tag="pp",
                                  name=f"gps{t}{w}")
                    for kc in range(NKC):
                        nc.tensor.matmul(
                            gps[:], lhs_of(w, kc), xt[:, kc, :],
                            start=(kc == 0), stop=(kc == NKC - 1))
                    proj_tail(w, gps)

                if t == 0:
                    load_masks(0, mtiles_all)
                    # o_proj weights: off the startup critical path
                    nc.sync.dma_start(
                        wo_sb[:], wo[:].rearrange("(t p) m -> p t m", p=128))
                # prefetch next x chunk (after proj: off the early DMA path)
                if t < 3:
                    xn_r = xT[:, ts0 + 512:ts0 + 1024].rearrange(
                        "(c p) t -> p c t", p=128)
                    for piece in range(4):
                        psl = slice(piece * 4, (piece + 1) * 4)
                        nc.sync.dma_start(xts[t + 1][:, psl, :],
                                          xn_r[:, psl, :])

                # -------- attention; o_proj lags one query chunk --------
                for qc in range(4 * t, 4 * t + 4):
                    q0 = qc * TQ
                    row = sched[qc]
                    if qc + 1 < NTC:
                        load_masks(qc + 1, mtiles_all)
                    mtiles = mtiles_all.pop(qc)
                    for g in range(NKVC):
                        o_ps = pop.tile([128, 4 * TQ], F32, tag="po",
                                        name=f"ops{qc}{g}")
                        for i, (kt, mi) in enumerate(row):
                            s_ps = psp.tile([128, 4 * TQ], F32, tag="ps",
                                            name=f"sps{qc}{g}{i}")
                            nc.tensor.matmul(
                                s_ps[:], krt[:, g, kt * 128:(kt + 1) * 128],
                                qrt[:, g, :, q0:q0 + TQ],
                                start=True, stop=True)
                            pt = ptp.tile([128, 4 * TQ], BF16, tag="pt")
                            nc.scalar.activation(pt[:], s_ps[:], Exp,
                                                 scale=SCALE)
                            if mi is not None:
                                # all-SBUF bf16: legal on Pool; split the
                                # two groups across DVE and Pool
                                if g == 0:
                                    nc.vector.tensor_mul(pt[:], pt[:],
                                                         mtiles[kt][:])
                                else:
                                    nc.gpsimd.tensor_mul(pt[:], pt[:],
                                                         mtiles[kt][:])
                            nc.tensor.matmul(
                                o_ps[:], vaug[:, g, kt, :], pt[:],
                                start=(i == 0), stop=(i == len(row) - 1))
                        zb = zbp.tile([64, 4 * TQ], F32, tag="zb")
                        with nc.allow_low_precision(
                                reason="softmax denominator reciprocal"):
                            nc.vector.reciprocal(zb[:], o_ps[64:128, :])
                        for hh in range(4):
                            fsl = slice(hh * TQ, (hh + 1) * TQ)
                            p0 = (hh % 2) * 64
                            nc.vector.tensor_mul(
                                or2t[p0:p0 + 64, 2 * g + hh // 2,
                                     q0:q0 + TQ],
                                o_ps[0:64, fsl], zb[:, fsl])
                    # o_proj for the previous token tile (lag 1 so DVE
                    # normalization of this qc overlaps PE o_proj work)
                    if qc > 0:
                        o_proj(qc - 1)
            o_proj(NTC - 1, last=True)

    nc.finalize()
    return nc


def _prep_inputs(x, rope_cos, rope_sin, doc_ids, Wq, Wk, Wv, Wo):
    import ml_dtypes
    BF = ml_dtypes.bfloat16
    x = np.asarray(x, np.float32)
    xTb = [np.ascontiguousarray(x[b].T).astype(BF) for b in range(B)]
    cosT = np.asarray(rope_cos, np.float32).T          # (32, S)
    sinT = np.asarray(rope_sin, np.float32).T
    cos128 = np.tile(np.concatenate([cosT, cosT], 0), (2, 1)).astype(BF)
    sin128 = np.tile(np.concatenate([sinT, -sinT], 0), (2, 1)).astype(BF)
    sched, mk = _schedule(doc_ids)
    Wq = np.asarray(Wq, np.float32)
    Wk = np.asarray(Wk, np.float32)
    Wv = np.asarray(Wv, np.float32)
    Wo = np.asarray(Wo, np.float32)
    in_maps = []
    for c in range(NCORES):
        b, qq = divmod(c, CPB)
        h0 = qq * HPC
        wq_c = np.ascontiguousarray(
            Wq[h0 * HD:(h0 + HPC) * HD].T).astype(BF)       # (DIM, 512)
        kv_cols = []
        for g in range(NKVC):
            kvh = qq * NKVC + g
            kv_cols.append(Wk[kvh * HD:(kvh + 1) * HD].T)   # (DIM, 64)
            kv_cols.append(Wv[kvh * HD:(kvh + 1) * HD].T)
        wkv_c = np.ascontiguousarray(
            np.concatenate(kv_cols, 1)).astype(BF)          # (DIM, 256)
        wo_c = np.ascontiguousarray(
            Wo[:, h0 * HD:(h0 + HPC) * HD].T).astype(BF)    # (512, DIM)
        in_maps.append({
            "xT": xTb[b], "wq": wq_c, "wkv": wkv_c, "wo": wo_c,
            "cos128": cos128, "sin128": sin128, "masks": mk[b],
        })
    return sched, mk, in_maps


def kernel(x, rope_cos, rope_sin, doc_ids, Wq, Wk, Wv, Wo):
    from concourse.bass_utils import run_bass_kernel_spmd
    sched, mk, in_maps = _prep_inputs(
        x, rope_cos, rope_sin, doc_ids, Wq, Wk, Wv, Wo)
    key = (tuple(tuple((kt, mi is not None) for kt, mi in row)
                 for row in sched), mk[0].shape[0])
    nc = _nc_cache.get(key)
    if nc is None:
        nc = _build_nc(sched, mk[0].shape[0])
        _nc_cache[key] = nc
    res = run_bass_kernel_spmd(nc, in_maps, core_ids=list(range(NCORES)))
    y = np.zeros((B, S, DIM), np.float32)
    for c in range(NCORES):
        y[c // CPB] += np.asarray(res.results[c]["y"], np.float32)
    return y
